# revision 1
# baseline (speedup 1.0000x reference)
"""ParabolicPool2D (max-plus pooling with per-channel parabolic kernel) on 8 trn2 cores.

out[b,c,ho,wo] = max_{ki,kj} f[b,c,2ho+ki-3,2wo+kj-3] + h[c,ki,kj]
with h[c,ki,kj] = -(z[ki]^2 + z[kj]^2) / (4 t[c]),  z = linspace(-2,3,7).

Separable: h[c,ki,kj] = a[c,ki] + a[c,kj], a = -z^2/(4t). Two 1D 7-tap
max-plus passes (W then H), each tap one scalar_tensor_tensor on DVE.

Sharding: batch-parallel, 2 images per core. Per core the (b,c) dim is 192;
to use all 128 partitions we split each image's H into two halves
(192 bc x 2 halves = 384 = 3 x 128 partition-passes). Each half loads a
3-row halo; out-of-image halo rows are -30000 pads so all compute ops are
uniform across partitions.

v2: fp16 datapath. ACT deinterleaves each f slab into even/odd fp16 column
tiles (fe/fo) with explicit -30000 pad columns, so all 7 horizontal taps are
uniform 112-wide stride-1 fp16 STT/TS ops (eligible for the packed 2-byte
DVE fast modes). g and the stage-2 accumulator are fp16; output is DMA'd as
fp16 and upcast to fp32 on the host (halves output DMA traffic).
"""

import os
import sys

sys.path.insert(0, "/opt/trn_rl_repo")

import numpy as np

from contextlib import ExitStack

from concourse import bacc, bass, mybir, tile
from concourse.bass_utils import run_bass_kernel_spmd

KS = 7
C = 96
B = 16
H = 224
W = 224
HO = 112
WO = 112
NCORES = 8
BC = (B // NCORES) * C  # 192 (b,c) rows per core
R = 117  # local g rows per half: 3 halo + 112 + 2
NEG = -30000.0  # pad; stays finite in fp16

# stage-1 taps k != 0: (k, parity, offset); src = (fe if parity else fo)[off:off+112]
S1_TAPS = [
    (1, 1, 0),
    (2, 0, 1),
    (3, 1, 1),
    (4, 0, 2),
    (5, 1, 2),
    (6, 0, 3),
]

# passes: list of groups (p0, p1, half, bc0)
PASSES = [
    [(0, 128, 0, 0)],
    [(0, 64, 0, 128), (64, 128, 1, 0)],
    [(0, 128, 1, 64)],
]
# half 0: local row r holds f row r-3 (valid local [3,117)), out rows [0,56)
# half 1: local row r holds f row r+109 (valid local [0,115)), out rows [56,112)
HALF_VALID = {0: (3, 117), 1: (0, 115)}
HALF_F0 = {0: -3, 1: 109}
HALF_HO0 = {0: 0, 1: 56}

SLABS = [(0, 30), (30, 59), (59, 88), (88, 117)]

_CACHE = {}


def _build(iters=1):
    nc = bacc.Bacc("TRN2", target_bir_lowering=False, debug=False)
    f32 = mybir.dt.float32
    f16 = mybir.dt.float16
    f_d = nc.dram_tensor("f", [BC, H, W], f32, kind="ExternalInput")
    bias_d = nc.dram_tensor("bias", [len(PASSES), 128, KS], f32, kind="ExternalInput")
    out_d = nc.dram_tensor("out", [BC, HO, WO], f16, kind="ExternalOutput")
    fa, ba, oa = f_d.ap(), bias_d.ap(), out_d.ap()

    add, mx = mybir.AluOpType.add, mybir.AluOpType.max

    with ExitStack() as ctx:
        tc = ctx.enter_context(tile.TileContext(nc))
        fin_pool = ctx.enter_context(tc.tile_pool(name="fin", bufs=3))
        eo_pool = ctx.enter_context(tc.tile_pool(name="eo", bufs=2))
        g_pool = ctx.enter_context(tc.tile_pool(name="g", bufs=1))
        out_pool = ctx.enter_context(tc.tile_pool(name="outp", bufs=2))
        bias_pool = ctx.enter_context(tc.tile_pool(name="bias", bufs=2))

        for t, groups in [(t, g) for _ in range(iters) for t, g in enumerate(PASSES)]:
            bias_t = bias_pool.tile([128, KS], f32)
            nc.sync.dma_start(bias_t[:], ba[t])
            bias16 = bias_pool.tile([128, KS], f16)
            nc.scalar.copy(bias16[:], bias_t[:])
            g = g_pool.tile([128, R, WO], f16)

            for rl0, rl1 in SLABS:
                rs = rl1 - rl0
                fin = fin_pool.tile([128, 30, W], f32)
                for p0, p1, half, bc0 in groups:
                    vlo, vhi = HALF_VALID[half]
                    lo, hi = max(rl0, vlo), min(rl1, vhi)
                    off = HALF_F0[half]
                    nc.sync.dma_start(
                        fin[p0:p1, lo - rl0 : hi - rl0, :],
                        fa[bc0 : bc0 + (p1 - p0), lo + off : hi + off, :],
                    )
                    if lo > rl0:
                        nc.gpsimd.memset(fin[p0:p1, 0 : lo - rl0, :], NEG)
                    if hi < rl1:
                        nc.gpsimd.memset(fin[p0:p1, hi - rl0 : rs, :], NEG)

                # ACT deinterleave + cast: fe[j]=f[2j-2], fo[j]=f[2j-3] (fp16)
                fe = eo_pool.tile([128, 30, 114], f16)
                fo = eo_pool.tile([128, 30, 115], f16)
                nc.scalar.copy(fe[:, 0:rs, 1:113], fin[:, 0:rs, 0:223:2])
                nc.scalar.copy(fo[:, 0:rs, 2:114], fin[:, 0:rs, 1:224:2])
                nc.gpsimd.memset(fe[:, 0:rs, 0:1], NEG)
                nc.gpsimd.memset(fe[:, 0:rs, 113:114], NEG)
                nc.gpsimd.memset(fo[:, 0:rs, 0:2], NEG)
                nc.gpsimd.memset(fo[:, 0:rs, 114:115], NEG)

                gs = g[:, rl0:rl1, :]
                # init tap k=0: g = fo[wo] + a0 (TS, fp32 scalar)
                nc.vector.tensor_scalar_add(gs, fo[:, 0:rs, 0:112], bias_t[:, 0:1])
                for k, par, off in S1_TAPS:
                    src = fe if par else fo
                    nc.vector.scalar_tensor_tensor(
                        gs,
                        src[:, 0:rs, off : off + 112],
                        bias16[:, k : k + 1],
                        gs,
                        add,
                        mx,
                    )

            out_t = out_pool.tile([128, 56, WO], f16)
            # stage-2 init k=0: out = g[2j] + a0
            nc.vector.tensor_scalar_add(out_t[:], g[:, 0:111:2, :], bias_t[:, 0:1])
            for k in (1, 2, 3, 4, 5, 6):
                nc.vector.scalar_tensor_tensor(
                    out_t[:],
                    g[:, k : k + 111 : 2, :],
                    bias16[:, k : k + 1],
                    out_t[:],
                    add,
                    mx,
                )
            for p0, p1, half, bc0 in groups:
                ho0 = HALF_HO0[half]
                nc.sync.dma_start(
                    oa[bc0 : bc0 + (p1 - p0), ho0 : ho0 + 56, :],
                    out_t[p0:p1, :, :],
                )
    nc.compile()
    return nc


def _bias_array(t: np.ndarray) -> np.ndarray:
    z = np.linspace(-2.0, 3.0, KS, dtype=np.float32)
    a = -(z[None, :] ** 2) / (4.0 * t[:, None].astype(np.float32))  # [C, KS]
    a_bc = np.tile(a, (B // NCORES, 1))  # [192, KS]
    out = np.empty((len(PASSES), 128, KS), dtype=np.float32)
    for t_i, groups in enumerate(PASSES):
        for p0, p1, _half, bc0 in groups:
            out[t_i, p0:p1] = a_bc[bc0 : bc0 + (p1 - p0)]
    return out


LAST_EXEC_NS = None


def _make_runner(nc):
    import jax
    from jax.experimental.shard_map import shard_map
    from jax.sharding import Mesh, NamedSharding, PartitionSpec

    from concourse import bass2jax

    bass2jax.install_neuronx_cc_hook()
    partition_name = nc.partition_id_tensor.name if nc.partition_id_tensor else None
    in_names, out_names, out_avals = [], [], []
    for alloc in nc.m.functions[0].allocations:
        if not isinstance(alloc, mybir.MemoryLocationSet):
            continue
        name = alloc.memorylocations[0].name
        if alloc.kind == "ExternalInput":
            if name != partition_name:
                in_names.append(name)
        elif alloc.kind == "ExternalOutput":
            out_names.append(name)
            out_avals.append(
                jax.core.ShapedArray(
                    tuple(alloc.tensor_shape), mybir.dt.np(alloc.dtype)
                )
            )
    n_params, n_outs = len(in_names), len(out_avals)
    all_names = list(in_names + out_names)
    if partition_name is not None:
        all_names.append(partition_name)
    all_names = tuple(all_names)
    donate = tuple(range(n_params, n_params + n_outs))

    def _body(*args):
        operands = list(args)
        if partition_name is not None:
            operands.append(bass2jax.partition_id_tensor())
        return tuple(
            bass2jax._bass_exec_p.bind(
                *operands,
                out_avals=tuple(out_avals),
                in_names=all_names,
                out_names=tuple(out_names),
                lowering_input_output_aliases=(),
                sim_require_finite=True,
                sim_require_nnan=True,
                nc=nc,
            )
        )

    mesh = Mesh(np.asarray(jax.devices()[:NCORES]), ("core",))
    sharded = jax.jit(
        shard_map(
            _body,
            mesh=mesh,
            in_specs=(PartitionSpec("core"),) * (n_params + n_outs),
            out_specs=(PartitionSpec("core"),) * n_outs,
            check_rep=False,
        ),
        donate_argnums=donate,
        keep_unused=True,
    )
    sh = NamedSharding(mesh, PartitionSpec("core"))
    return sharded, in_names, out_names, out_avals, sh


def _timed_run(nc, in_maps, ncalls=8):
    """Run nc on 8 cores with device-resident inputs; return per-call seconds
    (excluding input transfer) and core-0..7 outputs of the last call."""
    import time as _time

    import jax

    sharded, in_names, out_names, out_avals, sh = _make_runner(nc)
    concat_in = [
        np.concatenate([np.asarray(m[nm]) for m in in_maps], axis=0)
        for nm in in_names
    ]
    dev_in = [jax.device_put(x, sh) for x in concat_in]
    zero_sets = [
        [
            jax.device_put(
                np.zeros((NCORES * a.shape[0], *a.shape[1:]), a.dtype), sh
            )
            for a in out_avals
        ]
        for _ in range(ncalls + 1)
    ]
    out = sharded(*dev_in, *zero_sets[0])
    jax.block_until_ready(out)
    times = []
    for i in range(1, ncalls + 1):
        t0 = _time.perf_counter()
        out = sharded(*dev_in, *zero_sets[i])
        jax.block_until_ready(out)
        times.append(_time.perf_counter() - t0)
    outs = [
        {
            nm: np.asarray(out[i]).reshape(NCORES, *out_avals[i].shape)[c]
            for i, nm in enumerate(out_names)
        }
        for c in range(NCORES)
    ]
    return times, outs


def measure_hw_time(f: np.ndarray, t: np.ndarray, iters=9, ncalls=8):
    """Estimate per-invocation HW time via N-iteration differencing."""
    global LAST_EXEC_NS
    bias = _bias_array(np.asarray(t))
    f = np.ascontiguousarray(np.asarray(f, dtype=np.float32))
    per_core = B // NCORES
    in_maps = [
        {
            "f": np.ascontiguousarray(
                f[s * per_core : (s + 1) * per_core].reshape(BC, H, W)
            ),
            "bias": bias,
        }
        for s in range(NCORES)
    ]
    t1, _ = _timed_run(_build(1), in_maps, ncalls)
    tN, _ = _timed_run(_build(iters), in_maps, ncalls)
    hw_ns = (min(tN) - min(t1)) / (iters - 1) * 1e9
    LAST_EXEC_NS = int(hw_ns)
    return {
        "t1": t1,
        "tN": tN,
        "iters": iters,
        "hw_ns": hw_ns,
        "upper_bound_ns": min(t1) * 1e9,
    }


def kernel(f: np.ndarray, t: np.ndarray) -> np.ndarray:
    global LAST_EXEC_NS
    if "nc" not in _CACHE:
        _CACHE["nc"] = _build()
    nc = _CACHE["nc"]

    bias = _bias_array(np.asarray(t))
    f = np.ascontiguousarray(np.asarray(f, dtype=np.float32))
    per_core = B // NCORES
    in_maps = [
        {
            "f": np.ascontiguousarray(
                f[s * per_core : (s + 1) * per_core].reshape(BC, H, W)
            ),
            "bias": bias,
        }
        for s in range(NCORES)
    ]
    trace = os.environ.get("BASS_TRACE", "0") == "1"
    res = run_bass_kernel_spmd(nc, in_maps, core_ids=list(range(NCORES)), trace=trace)
    LAST_EXEC_NS = res.exec_time_ns

    out = np.empty((B, C, HO, WO), dtype=np.float32)
    for s in range(NCORES):
        out[s * per_core : (s + 1) * per_core] = res.results[s]["out"].reshape(
            per_core, C, HO, WO
        )
    return out

